# revision 1
# baseline (speedup 1.0000x reference)
"""Trainium2 Bass kernel for 2D-relative-bias multi-head attention.

Shapes (hardcoded): x [64, 16, 16, 512], 16 heads x 32 dim, S = 256.
Sharding: data-parallel over batch, 8 batches per core on 8 cores.

Per-core device pipeline (all matmuls bf16, fp32 PSUM accumulation):
  qT/kT = W^T @ x^T            [nd, tok]   (PE, K=c)
  v     = x @ Wv               [tok, nd]   (PE)
  logitsT[t,s] per head        (PE, K=32, 4-head row-packed via tile_position)
  E0 = exp(logitsT)            (ACT, PSUM->SBUF bf16)
  E  = E0 * exp(biasT)         (DVE, bias table precomputed on host)
  sums = 1^T E (replicated)    (PE, 4-head col-packed, all-ones lhsT)
  out_unT = V^T E              (PE, 4-head col-packed)
  R = 1/sums                   (DVE reciprocal_approx_fast)
  outT = out_unT * R           (DVE)
  final = outT^T @ Wo + o_b    (PE)
"""

import numpy as np
import ml_dtypes

try:
    import concourse.bass as bass
except ImportError:  # pragma: no cover
    import sys

    sys.path.insert(0, "/opt/trn_rl_repo")
    import concourse.bass as bass
from concourse import bacc

import concourse.mybir as mybir
import concourse.tile as tile
from concourse.bass_utils import run_bass_kernel_spmd

BF16 = mybir.dt.bfloat16
F32 = mybir.dt.float32
AF = mybir.ActivationFunctionType
OP = mybir.AluOpType

B, H, W, C = 64, 16, 16, 512
NH, D = 16, 32
S = H * W            # 256
NCORES = 8
BPC = B // NCORES    # 8 batches per core
TOK = BPC * S        # 2048 tokens per core
SCALE = D ** -0.5


def build_program(reps: int = 1, debug: bool = False, sections=('qkv', 'attn', 'sums', 'av', 'out'), with_qkbias: bool = False):
    nc = bacc.Bacc()
    xT_d = nc.dram_tensor("xT", [128, 4 * TOK], BF16, kind="ExternalInput")
    wqkvo_d = nc.dram_tensor("wqkvo", [128, 16 * 512], BF16, kind="ExternalInput")
    expb_d = nc.dram_tensor("expb", [128, 2 * NH * S], BF16, kind="ExternalInput")
    qb_d = nc.dram_tensor("qb", [1, 512], BF16, kind="ExternalInput")
    kb_d = nc.dram_tensor("kb", [1, 512], BF16, kind="ExternalInput")
    ones_r_d = nc.dram_tensor("ones_r", [1, 512], BF16, kind="ExternalInput")
    ones_c_d = nc.dram_tensor("ones_c", [128, 32], BF16, kind="ExternalInput")
    out_d = nc.dram_tensor("out", [TOK, 512], F32, kind="ExternalOutput")
    if debug:
        dbg_qT = nc.dram_tensor("dbg_qT", [128, TOK], F32, kind="ExternalOutput")
        dbg_kT = nc.dram_tensor("dbg_kT", [128, TOK], F32, kind="ExternalOutput")
        dbg_v = nc.dram_tensor("dbg_v", [128, 512], F32, kind="ExternalOutput")
        dbg_e0 = nc.dram_tensor("dbg_e0", [128, NH * S], F32, kind="ExternalOutput")
        dbg_e = nc.dram_tensor("dbg_e", [128, NH * S], F32, kind="ExternalOutput")
        dbg_s = nc.dram_tensor("dbg_s", [128, 1024], F32, kind="ExternalOutput")
        dbg_r = nc.dram_tensor("dbg_r", [128, 1024], F32, kind="ExternalOutput")
        dbg_ot = nc.dram_tensor("dbg_ot", [128, 256], F32, kind="ExternalOutput")

    with tile.TileContext(nc) as tc:
        import contextlib

        with contextlib.ExitStack() as ctx:
            wpool = ctx.enter_context(tc.tile_pool(name="wpool", bufs=1))
            xpool = ctx.enter_context(tc.tile_pool(name="xpool", bufs=1))
            qkpool = ctx.enter_context(tc.tile_pool(name="qkpool", bufs=1))
            epool = ctx.enter_context(tc.tile_pool(name="epool", bufs=3))
            rpool = ctx.enter_context(tc.tile_pool(name="rpool", bufs=2))
            otpool = ctx.enter_context(tc.tile_pool(name="otpool", bufs=8))
            fpool = ctx.enter_context(tc.tile_pool(name="fpool", bufs=3))
            dpool = ctx.enter_context(tc.tile_pool(name="dpool", bufs=1)) if debug else None
            pl_pool = ctx.enter_context(
                tc.tile_pool(name="pl", bufs=2, space="PSUM"))
            pa_pool = ctx.enter_context(
                tc.tile_pool(name="pa", bufs=1, space="PSUM"))
            ps_pool = ctx.enter_context(
                tc.tile_pool(name="ps", bufs=2, space="PSUM"))

            # ---- persistent constants ----
            wall = wpool.tile([128, 16 * 512], BF16, name="wall", tag="wall")
            nc.sync.dma_start(wall[:], wqkvo_d[:])
            wq = [wall[:, (3 * i + 0) * 512:(3 * i + 1) * 512] for i in range(4)]
            wk = [wall[:, (3 * i + 1) * 512:(3 * i + 2) * 512] for i in range(4)]
            wv = [wall[:, (3 * i + 2) * 512:(3 * i + 3) * 512] for i in range(4)]
            wo = [wall[:, (12 + i) * 512:(13 + i) * 512] for i in range(4)]
            expb_all = wpool.tile([128, 2 * NH * S], BF16, name="expb", tag="expb")
            nc.sync.dma_start(expb_all[:], expb_d[:])
            expb = [expb_all[:, t * NH * S:(t + 1) * NH * S] for t in range(2)]
            qb = wpool.tile([1, 512], BF16, name="qb", tag="qb")
            kb = wpool.tile([1, 512], BF16, name="kb", tag="kb")
            ones_r = wpool.tile([1, 512], BF16, name="ones_r", tag="ones_r")
            ones_c = wpool.tile([128, 32], BF16, name="ones_c", tag="ones_c")
            nc.sync.dma_start(qb[:], qb_d[:])
            nc.sync.dma_start(kb[:], kb_d[:])
            nc.sync.dma_start(ones_r[:], ones_r_d[:])
            nc.sync.dma_start(ones_c[:], ones_c_d[:])
            xT_all = xpool.tile([128, 4 * TOK], BF16, name="xT", tag="xT")
            nc.sync.dma_start(xT_all[:], xT_d[:])
            xT = [xT_all[:, i * TOK:(i + 1) * TOK] for i in range(4)]

            for _rep in range(reps):
                do = lambda s: s in sections
                # ---- phase 1: QKV projection chunk emitter (interleaved) ----
                qT = [qkpool.tile([128, TOK], BF16, name=f"qT{m}", tag=f"qT{m}") for m in range(4)]
                kT = [qkpool.tile([128, TOK], BF16, name=f"kT{m}", tag=f"kT{m}") for m in range(4)]
                v_sb = [qkpool.tile([128, 512], BF16, name=f"v{s}", tag=f"v{s}")
                        for s in range(TOK // 128)]

                def emit_qkv_chunk(nch):
                    """q,k projections for token chunk nch (512 tokens) + v for its 4 s-chunks."""
                    sl = slice(nch * 512, (nch + 1) * 512)
                    for wt, bt, dst in ((wq, qb, qT), (wk, kb, kT)):
                        for m in range(4):
                            ps = ps_pool.tile([128, 512], F32, name="ps", tag="ps")
                            for kc in range(4):
                                nc.tensor.matmul(
                                    ps[:, :512],
                                    wt[kc][:, m * 128:(m + 1) * 128],
                                    xT[kc][:, sl],
                                    start=(kc == 0),
                                    stop=(kc == 3 and not with_qkbias))
                            if with_qkbias:
                                nc.tensor.matmul(
                                    ps[:, :512],
                                    bt[0:1, m * 128:(m + 1) * 128],
                                    ones_r[0:1, :512],
                                    start=False, stop=True)
                            nc.vector.tensor_copy(dst[m][:, sl], ps[:, :512])
                    for sch in range(nch * 4, (nch + 1) * 4):
                        ps = ps_pool.tile([128, 512], F32, name="ps", tag="ps")
                        for kc in range(4):
                            nc.tensor.matmul(
                                ps[:, :512],
                                xT[kc][:, sch * 128:(sch + 1) * 128],
                                wv[kc][:, :512],
                                start=(kc == 0), stop=(kc == 3))
                        nc.vector.tensor_copy(v_sb[sch][:], ps[:, :512])

                # ---- phase 2: attention, software-pipelined over batches ----
                def stage_front(b):
                    """logits -> exp -> bias-mul; returns E tiles for batch b."""
                    ssl = slice(b * S, (b + 1) * S)
                    E = []
                    for tch in range(2):
                        e0 = epool.tile([128, NH * S], BF16, name="e0", tag="e0", bufs=2)
                        tsl = slice(b * S + tch * 128, b * S + tch * 128 + 128)
                        for hg in range(4):
                            for hp in range(2):
                                pl = pl_pool.tile([128, 1024], F32, name="pl", tag="pl")
                                for hi in range(2):
                                    hl = 2 * hp + hi
                                    nc.tensor.matmul(
                                        pl[:, hi * 512:hi * 512 + 256],
                                        kT[hg][32 * hl:32 * hl + 32, tsl],
                                        qT[hg][32 * hl:32 * hl + 32, ssl],
                                        start=True, stop=True,
                                        tile_position=(32 * hl, 0))
                                pl_v = pl.rearrange("p (h x) -> p h x", h=2)[:, :, :256]
                                n0 = 4 * hg + 2 * hp
                                e0_v = e0[:, n0 * 256:(n0 + 2) * 256].rearrange(
                                    "p (h x) -> p h x", h=2)
                                nc.scalar.activation(e0_v, pl_v, AF.Exp)
                        e = epool.tile([128, NH * S], BF16, name="e", tag="e", bufs=4)
                        nc.vector.tensor_tensor(e[:], e0[:], expb[tch][:], OP.mult)
                        E.append(e)
                    return E

                def stage_back(b, E):
                    """sums -> recip -> AV -> norm -> outproj -> DMA for batch b."""
                    if not do('sums'):
                        return
                    r = rpool.tile([128, 1024], F32, name="r", tag="r")
                    for sh in range(2):
                        psum_s = ps_pool.tile([128, 512], F32, name="ps", tag="ps")
                        for hg in (2 * sh, 2 * sh + 1):
                            for j in range(4):
                                n = 4 * hg + j
                                for tch in range(2):
                                    nc.tensor.matmul(
                                        psum_s[32 * j:32 * j + 32,
                                               (hg - 2 * sh) * 256:
                                               (hg - 2 * sh + 1) * 256],
                                        ones_c[:, :32],
                                        E[tch][:, n * 256:(n + 1) * 256],
                                        start=(tch == 0), stop=(tch == 1),
                                        tile_position=(0, 32 * j))
                        nc.vector.reciprocal_approx_fast(
                            r[:, sh * 512:(sh + 1) * 512], psum_s[:])
                    if not do('av'):
                        return
                    pa = pa_pool.tile([128, 1024], F32, name="pa", tag="pa")
                    for hg in range(4):
                        for j in range(4):
                            n = 4 * hg + j
                            for tch in range(2):
                                nc.tensor.matmul(
                                    pa[32 * j:32 * j + 32,
                                       hg * 256:(hg + 1) * 256],
                                    v_sb[2 * b + tch][:, n * 32:(n + 1) * 32],
                                    E[tch][:, n * 256:(n + 1) * 256],
                                    start=(tch == 0), stop=(tch == 1),
                                    tile_position=(0, 32 * j))
                    ot = otpool.tile([128, 1024], BF16, name="ot", tag="ot")
                    nc.vector.tensor_tensor(ot[:], pa[:], r[:], OP.mult)
                    if not do('out'):
                        return
                    po = pa_pool.tile([128, 1024], F32, name="po", tag="pa")
                    for sch in range(2):
                        for hg in range(4):
                            nc.tensor.matmul(
                                po[:, sch * 512:(sch + 1) * 512],
                                ot[:, hg * 256 + sch * 128:
                                   hg * 256 + (sch + 1) * 128],
                                wo[hg][:, :512],
                                start=(hg == 0), stop=(hg == 3))
                    fs = fpool.tile([128, 1024], F32, name="f", tag="f")
                    nc.vector.tensor_copy(fs[:], po[:])
                    dst = out_d[b * S:(b + 1) * S, :].rearrange(
                        "(c p) w -> p c w", p=128)
                    nc.sync.dma_start(dst, fs.rearrange("p (c w) -> p c w", c=2))

                emit_qkv_chunk(0)
                if do('attn'):
                    prev = None
                    for b in range(BPC):
                        E = stage_front(b)
                        if b % 2 == 0 and b // 2 + 1 < 4:
                            emit_qkv_chunk(b // 2 + 1)
                        if prev is not None:
                            stage_back(prev[0], prev[1])
                        prev = (b, E)
                    stage_back(prev[0], prev[1])
                else:
                    for nch in range(1, 4):
                        emit_qkv_chunk(nch)

    nc.compile()
    return nc


def _bias_tables(rel_emb):
    """expb[tch, t_local, n*256+s] = exp(bias[n, s, t]) with t = tch*128+t_local."""
    idx = np.arange(H)
    rel = idx[None, :] - idx[:, None] + (H - 1)          # [a, b] -> b - a + 15
    # bias[n, s, t] = rel_emb[n, th-sh+15, tw-sw+15]; biasT[n, t, s] = bias[n, s, t]
    rh = rel[:, :]                                        # [sh, th]
    biasT = rel_emb[:, rh.T[:, None, :, None], rel.T[None, :, None, :]]
    # biasT[n, th, tw, sh, sw] = rel_emb[n, th-sh+15, tw-sw+15]
    biasT = biasT.reshape(NH, S, S)                       # [n, t, s]
    expb = np.exp(biasT.astype(np.float64)).astype(np.float32)
    expb = np.ascontiguousarray(np.transpose(expb, (1, 0, 2)))  # [t, n, s]
    expb = expb.reshape(2, 128, NH * S).transpose(1, 0, 2).reshape(128, 2 * NH * S)
    return np.ascontiguousarray(expb).astype(ml_dtypes.bfloat16)


_CACHE = {}


def _get_program(key=1):
    if isinstance(key, tuple):
        reps, with_qkbias = key
    else:
        reps, with_qkbias = key, False
    k = (reps, with_qkbias)
    if k not in _CACHE:
        _CACHE[k] = build_program(reps, with_qkbias=with_qkbias)
    return _CACHE[k]


def make_in_maps(**inputs):
    x = np.asarray(inputs["x"], np.float32)
    q_w = np.asarray(inputs["q_w"], np.float32).reshape(C, NH * D)
    k_w = np.asarray(inputs["k_w"], np.float32).reshape(C, NH * D)
    v_w = np.asarray(inputs["v_w"], np.float32).reshape(C, NH * D)
    o_w = np.asarray(inputs["o_w"], np.float32).reshape(NH * D, C)
    q_b = np.asarray(inputs["q_b"], np.float32).reshape(NH * D)
    k_b = np.asarray(inputs["k_b"], np.float32).reshape(NH * D)
    v_b = np.asarray(inputs["v_b"], np.float32).reshape(NH * D)
    o_b = np.asarray(inputs["o_b"], np.float32).reshape(C)
    rel_emb = np.asarray(inputs["rel_emb"], np.float32)

    bf = ml_dtypes.bfloat16
    wq_s = (q_w * SCALE).reshape(4, 128, 512)
    wk_s = k_w.reshape(4, 128, 512)
    wv_s = v_w.reshape(4, 128, 512)
    wo_s = o_w.reshape(4, 128, 512)
    blocks = []
    for i in range(4):
        blocks += [wq_s[i], wk_s[i], wv_s[i]]
    blocks += [wo_s[i] for i in range(4)]
    wqkvo = np.ascontiguousarray(np.concatenate(blocks, axis=1)).astype(bf)
    expb = _bias_tables(rel_emb)
    qb = (q_b * SCALE).reshape(1, 512).astype(bf)
    kb = k_b.reshape(1, 512).astype(bf)
    ones_r = np.ones((1, 512), bf)
    ones_c = np.ones((128, 32), bf)

    in_maps = []
    for ci in range(NCORES):
        xc = x[ci * BPC:(ci + 1) * BPC].reshape(TOK, C)
        xT = np.ascontiguousarray(
            xc.T.reshape(4, 128, TOK).transpose(1, 0, 2).reshape(128, 4 * TOK)
        ).astype(bf)
        in_maps.append(dict(
            xT=xT, wqkvo=wqkvo, expb=expb,
            qb=qb, kb=kb, ones_r=ones_r, ones_c=ones_c))
    return in_maps


def kernel(**inputs):
    q_b = np.asarray(inputs["q_b"], np.float32).reshape(NH * D)
    k_b = np.asarray(inputs["k_b"], np.float32).reshape(NH * D)
    v_b = np.asarray(inputs["v_b"], np.float32).reshape(NH * D)
    o_b = np.asarray(inputs["o_b"], np.float32).reshape(C)
    o_w = np.asarray(inputs["o_w"], np.float32).reshape(NH * D, C)
    with_qkbias = bool(np.any(q_b) or np.any(k_b))
    nc = _get_program((1, with_qkbias))
    in_maps = make_in_maps(**inputs)
    res = run_bass_kernel_spmd(nc, in_maps, core_ids=list(range(NCORES)))
    outs = [res.results[ci]["out"].reshape(BPC, S, C) for ci in range(NCORES)]
    out = np.concatenate(outs, axis=0).astype(np.float32)
    # v_b rides through attention as a constant (rows of attn sum to 1); o_b is affine
    const = (v_b @ o_w) + o_b
    if np.any(const):
        out = out + const[None, None, :]
    return out

